# revision 1
# baseline (speedup 1.0000x reference)
"""Trainium2 Bass kernel for a batch-hard contrastive loss.

Math (verified against the reference formulation offline):
  d2[i,j]  = ||x_i||^2 + ||x_j||^2 - 2 x_i.x_j
  dist     = sqrt(max(d2, 0) + 1e-12)
  hardest_positive[i] = max_{j: same class}    dist[i,j]   (diag harmless: d2~0)
  hardest_negative[i] = min_{j: other class}   dist[i,j]
  loss = mean(hardest_positive) + mean(relu(margin - hardest_negative))

The (dist + col_max * not_negative) term in the reference never wins its min
(triangle inequality, margin > 5.7 on these inputs), so hardest_negative
reduces to the plain masked min. sqrt/clamp are monotone, so both masked
reductions run on d2 and only the [N] results get sqrt'ed.

Device strategy (8 cores, row-parallel, 512 rows each):
  One augmented matmul per [128,512] tile computes
      PSUM = x_i.x_j - sq_j/2 - (BIG/2)*same(i,j)
  via extra contraction rows: [-sq_j/2 (hi+lo split); -onehot(label_j)] on the
  moving side against [1; 1; (BIG/2)*onehot(label_i)] on the stationary side.
  Then  -2*PSUM + sq_i = d2 + BIG*same  =: neg_cand, and
      row_max(neg_cand) - BIG -> hardest-positive d2
      row_min(neg_cand)       -> hardest-negative d2
  Since -2x+c is monotone decreasing, the per-tile work is just a PSUM
  reduce_min and reduce_max on the vector engine; the -2/+bias affine step is
  applied to the [128,1] reduction results on the scalar engine.
"""

import numpy as np
from contextlib import ExitStack

N, D, NCLASS = 4096, 512, 64
NCORES = 8
RPC = N // NCORES          # rows per core = 512
MARGIN = 0.5
BIG = 32768.0
KROWS = D + 2 + NCLASS     # 512 x-rows + 2 sq-rows + 64 one-hot rows = 578
K_TILES = [(0, 128), (128, 128), (256, 128), (384, 128), (512, KROWS - 512)]
N_RT = RPC // 128          # 4 row tiles per core
N_CT = N // 512            # 8 col tiles

# Possible further optimization (not applied): sort rows by label and rotate
# each core's rhs columns by -core*RPC; the same-class block of every row
# tile then lands in a static set of 1-2 col tiles, letting the max-reduce
# (hardest-positive path) be skipped on the other ~26 of 32 tiles per core
# (~40% less DVE work; the min-reduce still needs all columns).

_CACHE = {}


def _fp32r_trunc(x: np.ndarray) -> np.ndarray:
    """Round fp32 to fp32r (tf32-style: low 13 mantissa bits zeroed).

    Round-to-nearest-even before masking: the HW only requires the low bits to
    be zero, and RNE halves the error vs plain truncation."""
    b = np.ascontiguousarray(x, dtype=np.float32).view(np.uint32).copy()
    b += np.uint32(0x0FFF) + ((b >> np.uint32(13)) & np.uint32(1))
    b &= np.uint32(0xFFFFE000)
    return b.view(np.float32)


def _build_nc():
    import concourse.bass as bass
    import concourse.tile as tile
    from concourse import bacc, mybir

    F32 = mybir.dt.float32
    R32 = mybir.dt.float32r
    AX = mybir.AxisListType.X
    OP = mybir.AluOpType

    # Bacc (not plain Bass): its compile() pass splits multi-wait instructions
    # into chains — walrus codegen allows at most one sync wait per instruction.
    nc = bacc.Bacc(None, target_bir_lowering=False)
    rhs_d = nc.dram_tensor("rhs_aug", [KROWS, N], R32, kind="ExternalInput")
    lhs_d = nc.dram_tensor("lhs_aug", [KROWS, RPC], R32, kind="ExternalInput")
    out_d = nc.dram_tensor("out", [128, 2 * N_RT], F32, kind="ExternalOutput")

    with tile.TileContext(nc) as tc, ExitStack() as ctx:
        const = ctx.enter_context(tc.tile_pool(name="const", bufs=1))
        psum = ctx.enter_context(
            tc.tile_pool(name="psum", bufs=8, space=bass.MemorySpace.PSUM)
        )
        stats = ctx.enter_context(tc.tile_pool(name="stats", bufs=1))

        # --- resident loads -------------------------------------------------
        rhs_sb = {}
        for kt, (k0, kp) in enumerate(K_TILES):
            for ct in range(N_CT):
                t = const.tile([kp, 512], R32, tag=f"rhs_{kt}_{ct}")
                nc.sync.dma_start(t[:], rhs_d[k0 : k0 + kp, ct * 512 : (ct + 1) * 512])
                rhs_sb[(kt, ct)] = t
        lhs_sb = {}
        for kt, (k0, kp) in enumerate(K_TILES):
            t = const.tile([kp, RPC], R32, tag=f"lhs_{kt}")
            nc.sync.dma_start(t[:], lhs_d[k0 : k0 + kp, :])
            lhs_sb[kt] = t
        out_sb = stats.tile([128, 2 * N_RT], F32, tag="out")

        # --- main loop ------------------------------------------------------
        for rt in range(N_RT):
            mn8 = stats.tile([128, N_CT], F32, tag=f"mn_{rt}")
            mx8 = stats.tile([128, N_CT], F32, tag=f"mx_{rt}")
            for ct in range(N_CT):
                ps = psum.tile([128, 512], F32, tag="ps")
                for kt in range(len(K_TILES)):
                    nc.tensor.matmul(
                        ps[:],
                        lhsT=lhs_sb[kt][:, rt * 128 : (rt + 1) * 128],
                        rhs=rhs_sb[(kt, ct)][:],
                        start=(kt == 0),
                        stop=(kt == len(K_TILES) - 1),
                    )
                nc.vector.tensor_reduce(mn8[:, ct : ct + 1], ps[:], axis=AX, op=OP.min)
                nc.vector.tensor_reduce(mx8[:, ct : ct + 1], ps[:], axis=AX, op=OP.max)
            nc.vector.tensor_reduce(
                out_sb[:, rt : rt + 1], mn8[:], axis=AX, op=OP.min
            )
            nc.vector.tensor_reduce(
                out_sb[:, N_RT + rt : N_RT + rt + 1], mx8[:], axis=AX, op=OP.max
            )
        # Funnel through one DVE copy so the output DMA has a single producer
        # (a DMA waiting on 8 reduce writes exceeds the per-instruction
        # sync-wait limit in walrus codegen).
        out2_sb = stats.tile([128, 2 * N_RT], F32, tag="out2")
        nc.vector.tensor_copy(out2_sb[:], out_sb[:])
        # gpsimd (SWDGE) for the store: the shared HWDGE queue would add a
        # second queue-credit wait and DMA instructions only support one.
        nc.gpsimd.dma_start(out_d[:], out2_sb[:])
    nc.compile()
    return nc


def _prep_inputs(feature, label):
    X = np.ascontiguousarray(np.asarray(feature), dtype=np.float32)
    lab = np.asarray(label).astype(np.int64)
    sq64 = (X.astype(np.float64) ** 2).sum(1)
    sq = sq64.astype(np.float32)
    onehot = (lab[:, None] == np.arange(NCLASS)[None, :]).astype(np.float32)

    half_neg_sq = (-0.5 * sq64).astype(np.float32)
    hi = _fp32r_trunc(half_neg_sq)
    lo = _fp32r_trunc(
        (half_neg_sq.astype(np.float64) - hi.astype(np.float64)).astype(np.float32)
    )

    rhs_aug = np.empty((KROWS, N), np.float32)
    rhs_aug[:D] = _fp32r_trunc(X).T
    rhs_aug[D] = hi
    rhs_aug[D + 1] = lo
    rhs_aug[D + 2 :] = -onehot.T

    lhs_full = np.empty((KROWS, N), np.float32)
    lhs_full[:D] = _fp32r_trunc(X).T
    lhs_full[D] = 1.0
    lhs_full[D + 1] = 1.0
    lhs_full[D + 2 :] = (BIG / 2.0) * onehot.T

    in_maps = []
    for m in range(NCORES):
        rows = slice(m * RPC, (m + 1) * RPC)
        in_maps.append(
            {
                "rhs_aug": rhs_aug,
                "lhs_aug": np.ascontiguousarray(lhs_full[:, rows]),
            }
        )
    return in_maps, sq64


def _gather(results, sq64):
    """out[:, rt] = row-min of PSUM, out[:, N_RT+rt] = row-max of PSUM, where
    PSUM = x_i.x_j - sq_j/2 - (BIG/2)*same.  Undo the affine map on host."""
    rmin = np.empty(N)
    rmax = np.empty(N)
    for m, r in enumerate(results):
        o = np.asarray(r["out"], np.float64)
        rows = slice(m * RPC, (m + 1) * RPC)
        rmin[rows] = o[:, :N_RT].T.reshape(-1)
        rmax[rows] = o[:, N_RT:].T.reshape(-1)
    hp_d2 = -2.0 * rmin + sq64 - BIG  # max over same-class of d2
    hn_d2 = -2.0 * rmax + sq64       # min over negatives of d2
    hp = np.sqrt(np.maximum(hp_d2, 0.0) + 1e-12)
    hn = np.sqrt(np.maximum(hn_d2, 0.0) + 1e-12)
    p_loss = hp.mean()
    n_loss = np.maximum(MARGIN - hn, 0.0).mean()
    return np.asarray(p_loss + n_loss, dtype=np.float32)


def kernel(feature, label):
    from concourse.bass_utils import run_bass_kernel_spmd

    if "nc" not in _CACHE:
        _CACHE["nc"] = _build_nc()
    nc = _CACHE["nc"]
    in_maps, sq64 = _prep_inputs(feature, label)
    rr = run_bass_kernel_spmd(nc, in_maps, list(range(NCORES)))
    return _gather(rr.results, sq64)

